# revision 53
# baseline (speedup 1.0000x reference)
"""Trainium2 Bass kernel for nn_DiagonalSSM (token-mix -> 2x [SAGE + diagonal SSM scan] -> proj).

Sharding: nodes (V) split across 8 cores. Message passing via per-core
dst-sorted edge lists; source features fetched with dma_gather from an
AllGathered node-major table; segment-sum on the tensor engine with a valued
one-hot (inverse-degree folded in) as the moving operand (32-wide dst blocks).

Layer-0 SSM scan on DVE (per-k lam mult + per-k2-pair h add); relu produces
fp8 tiles (split ACT/DVE by knob) consumed by DoubleRow fp8 matmuls against
pair-packed w_mix (4x fewer PE cycles than bf16 in the cost model).

Layer-1 only needs the final scan step: s7 = sum_t lam^(7-t) h1_t is computed
via PE diag-matmuls accumulating in PSUM over all t (h1 staged to DRAM during
layer-0's scan, reloaded into the then-dead state buffer), then
relu -> fp8 -> DoubleRow against folded w_mix@w_out to emit the output
directly (w_res1/biases folded through w_out on the host).
"""

import contextlib

import numpy as np
import ml_dtypes

import concourse.bacc as bacc
import concourse.bass as bass
import concourse.mybir as mybir
import concourse.tile as tile
from concourse.bass_utils import run_bass_kernel_spmd

BF16 = ml_dtypes.bfloat16
FP8 = ml_dtypes.float8_e4m3fn

NCORES = 8
BL = 64                      # dst-block width for the one-hot segment sum
GCH = 8                      # chunks (of 128 idxs) per dma_gather call
ER = 4                       # 2^ER scale on relu(state) before fp8 quant
EW = 7                       # 2^EW scale on w_mix before fp8 quant
RSPLIT = 7                  # k2 pairs with k2 < RSPLIT: relu on ACT; else DVE


class Cfg:
    def __init__(self, T=8, V=10000, E=100000, CIN=128, H=256, DS=16, CO=64):
        self.T, self.V, self.E = T, V, E
        self.CIN, self.H, self.DS, self.CO = CIN, H, DS, CO
        self.VLOC = V // NCORES                      # real nodes per core
        self.VL = ((self.VLOC + 127) // 128) * 128   # padded local nodes
        self.NB = self.VL // BL                      # dst blocks
        self.ncb = None                              # chunks per block (set by prep)
        self.VCS = []                                # v windows (<=512)
        off = 0
        while off < self.VL:
            w = min(512, self.VL - off)
            self.VCS.append((off, w))
            off += w
        self.K = (H * DS) // 128                     # state k-tiles
        self.K2 = self.K // 2                        # DoubleRow pairs
        self.MH = H // 128                           # output chunks of H
        self.CC = {0: max(1, CIN // 128), 1: H // 128}
        self.tail_tmin = [0] * self.K                # first t kept per k tile


# ----------------------------------------------------------------------------
# host-side preparation
# ----------------------------------------------------------------------------

def prep_edges(cfg, edge_index):
    T = cfg.T
    VLOC, VL, NB = cfg.VLOC, cfg.VL, cfg.NB
    ei = np.asarray(edge_index)
    src_all, dst_all = ei[:, 0, :].astype(np.int64), ei[:, 1, :].astype(np.int64)

    buckets = [[[None] * NB for _ in range(T)] for _ in range(NCORES)]
    deg = np.zeros((T, NCORES, VL), np.float32)
    for t in range(T):
        s_t, d_t = src_all[t], dst_all[t]
        core = np.minimum(d_t // VLOC, NCORES - 1)
        for c in range(NCORES):
            m = core == c
            s_c, d_c = s_t[m], d_t[m] - c * VLOC
            np.add.at(deg[t, c], d_c, 1.0)
            b_c = d_c // BL
            order = np.argsort(b_c, kind="stable")
            s_c, d_c, b_c = s_c[order], d_c[order], b_c[order]
            bounds = np.searchsorted(b_c, np.arange(NB + 1))
            for b in range(NB):
                lo, hi = bounds[b], bounds[b + 1]
                buckets[c][t][b] = (s_c[lo:hi], d_c[lo:hi])

    ncb = 1
    for c in range(NCORES):
        for t in range(T):
            for b in range(NB):
                ncb = max(ncb, (len(buckets[c][t][b][0]) + 127) // 128)
    cfg.ncb = ncb
    nchunk = NB * ncb
    invdeg = 1.0 / np.maximum(deg, 1.0)

    src_rows = np.zeros((NCORES, T, nchunk, 128), np.int16)
    scol = np.zeros((NCORES, T, nchunk, 128), np.float32)
    sval = np.zeros((NCORES, T, nchunk, 128), np.float32)
    for c in range(NCORES):
        for t in range(T):
            for b in range(NB):
                s_b, d_b = buckets[c][t][b]
                n = len(s_b)
                pad = ncb * 128 - n
                rows = (s_b // VLOC) * VL + (s_b % VLOC)
                rows = np.concatenate([rows, np.zeros(pad, np.int64)])
                col = np.concatenate([d_b - b * BL, np.zeros(pad, np.int64)])
                val = np.concatenate([invdeg[t, c][d_b], np.zeros(pad, np.float32)])
                cs = b * ncb
                src_rows[c, t, cs:cs + ncb] = rows.reshape(ncb, 128).astype(np.int16)
                scol[c, t, cs:cs + ncb] = col.reshape(ncb, 128)
                sval[c, t, cs:cs + ncb] = val.reshape(ncb, 128)
    return src_rows, scol, sval


def pack_gather_idx(cfg, src_rows):
    # [T, ngrp, 128, GCH*8]: GCH chunks of 128 idxs per dma_gather call,
    # idxs wrapped into 16 partitions and replicated x8.
    T = cfg.T
    nchunk = src_rows.shape[1]
    ngrp = (nchunk + GCH - 1) // GCH
    out = np.zeros((T, ngrp, 128, GCH * 8), np.int16)
    for t in range(T):
        for g in range(ngrp):
            flat = src_rows[t, g * GCH:(g + 1) * GCH].reshape(-1)
            buf = np.zeros(GCH * 128, np.int16)
            buf[:len(flat)] = flat
            out[t, g] = np.tile(buf.reshape(-1, 16).T, (8, 1))
    return out


def reorder_ssm(a_log, B, w_mix, H, DS):
    # state row order jd = j*H + i (j in DS, i in H); k-tile k = jd//128.
    # Rows are then permuted within each (m = i//128, p = i%128) class so
    # tile rank tau holds the tau-th largest lam -- late tiles have tiny lam
    # and their early-t contributions to s7 can be skipped in the tail.
    lam = np.exp(-np.exp(np.asarray(a_log, np.float64))).astype(np.float32)
    lam_r = lam.T.reshape(-1)                                 # [H*DS], jd order
    B_r = np.asarray(B, np.float32).T.reshape(-1)
    wm = np.asarray(w_mix, np.float32)                        # [H*DS, H], row = i*DS + j
    j_idx, i_idx = np.divmod(np.arange(H * DS), H)
    w_mix_r = wm[i_idx * DS + j_idx]                          # row jd = j*H + i
    K = (H * DS) // 128
    NJ = DS
    perm = np.zeros(H * DS, np.int64)                         # new row -> old jd
    for m in range(H // 128):
        for p in range(128):
            i = m * 128 + p
            jds = np.arange(NJ) * H + i
            order = np.argsort(-lam_r[jds], kind="stable")    # lam desc
            for tau in range(NJ):
                knew = m + 2 * tau
                perm[knew * 128 + p] = jds[order[tau]]
    lam_r = lam_r[perm]
    B_r = B_r[perm]
    w_mix_r = w_mix_r[perm]
    return lam_r.reshape(K, 128), B_r.reshape(K, 128), w_mix_r


# ----------------------------------------------------------------------------
# device program
# ----------------------------------------------------------------------------

def build_program(cfg, sim1=False):
    T, VL, CIN, H, DS, CO = cfg.T, cfg.VL, cfg.CIN, cfg.H, cfg.DS, cfg.CO
    K, K2, MH = cfg.K, cfg.K2, cfg.MH
    nchunk = cfg.NB * cfg.ncb
    ngrp = (nchunk + GCH - 1) // GCH
    fp32, bf16, i16 = mybir.dt.float32, mybir.dt.bfloat16, mybir.dt.int16
    fp8 = mybir.dt.float8e4
    AT = mybir.ActivationFunctionType
    OP = mybir.AluOpType
    PM = mybir.MatmulPerfMode

    ndev = 1 if sim1 else NCORES
    nc = bacc.Bacc("TRN2", target_bir_lowering=False, debug=False,
                   num_devices=ndev)

    xs_in = nc.dram_tensor("xs_in", [T, CIN, VL], fp32, kind="ExternalInput")
    idx_in = nc.dram_tensor("idx_in", [T, ngrp, 128, GCH * 8], i16, kind="ExternalInput")
    s_in = nc.dram_tensor("s_in", [T, 128, nchunk * BL], bf16, kind="ExternalInput")
    s8_in = nc.dram_tensor("s8_in", [T, 128, nchunk * BL], fp8, kind="ExternalInput")
    wpre_diag_in = nc.dram_tensor("wpre_diag_in", [3, CIN, CIN], bf16, kind="ExternalInput")
    bpre_in = nc.dram_tensor("bpre_in", [CIN, 1], fp32, kind="ExternalInput")
    ident_in = nc.dram_tensor("ident_in", [128, 128], bf16, kind="ExternalInput")
    lam_in = nc.dram_tensor("lam_in", [2, 128, K], fp32, kind="ExternalInput")
    sgn_in = nc.dram_tensor("sgn_in", [2, 128, K], fp32, kind="ExternalInput")
    bsg_in = nc.dram_tensor("bsg_in", [2, 128, MH], fp32, kind="ExternalInput")
    cvs_in = nc.dram_tensor("cvs_in", [128, T, K], fp32, kind="ExternalInput")
    wmix0_in = nc.dram_tensor("wmix0_in", [K2, 128, 2, H], fp8, kind="ExternalInput")
    wmo_in = nc.dram_tensor("wmo_in", [K2, 128, 2, CO], fp8, kind="ExternalInput")
    wsage_in = nc.dram_tensor("wsage_in", [2, 2, 2, 128, H], bf16, kind="ExternalInput")
    wres0_in = nc.dram_tensor("wres0_in", [128, H], bf16, kind="ExternalInput")
    wro_in = nc.dram_tensor("wro_in", [2, 128, CO], bf16, kind="ExternalInput")
    bmr_in = nc.dram_tensor("bmr_in", [128, MH], fp32, kind="ExternalInput")
    bout_in = nc.dram_tensor("bout_in", [64, 1], fp32, kind="ExternalInput")

    out_fm = nc.dram_tensor("out_fm", [CO, VL], fp32, kind="ExternalOutput")

    x0_T = nc.dram_tensor("x0_T", [T, CIN, VL], bf16)
    h1_T = nc.dram_tensor("h1_T", [T, 2, 128, VL], bf16)
    x0_nm = nc.dram_tensor("x0_nm", [T, VL, CIN], bf16)
    x1_nm = nc.dram_tensor("x1_nm", [T, VL, H], fp8)
    x0_full = nc.dram_tensor("x0_full", [T, NCORES * VL, CIN], bf16,
                             addr_space="Shared")
    x1_full = nc.dram_tensor("x1_full", [T, NCORES * VL, H], fp8,
                             addr_space="Shared")

    rg = [list(range(NCORES))]

    with tile.TileContext(nc) as tc, contextlib.ExitStack() as ctx:
        wpool = ctx.enter_context(tc.tile_pool(name="weights", bufs=1))
        spool = ctx.enter_context(tc.tile_pool(name="state", bufs=1))
        gpool = ctx.enter_context(tc.tile_pool(name="gather", bufs=5))
        opool = ctx.enter_context(tc.tile_pool(name="oh", bufs=8))
        ipool = ctx.enter_context(tc.tile_pool(name="ip", bufs=4))
        xpool = ctx.enter_context(tc.tile_pool(name="xt", bufs=2))
        mpool = ctx.enter_context(tc.tile_pool(name="misc", bufs=1))
        h0pool = ctx.enter_context(tc.tile_pool(name="hb0", bufs=2))
        h1pool = ctx.enter_context(tc.tile_pool(name="hb1", bufs=1))
        m0pool = ctx.enter_context(tc.tile_pool(name="mb0", bufs=1))
        m1pool = ctx.enter_context(tc.tile_pool(name="mb1", bufs=1))
        opoolx = ctx.enter_context(tc.tile_pool(name="outp", bufs=2))
        npool = ctx.enter_context(tc.tile_pool(name="nm", bufs=1))
        rpool = ctx.enter_context(tc.tile_pool(name="rl", bufs=4))
        dpool = ctx.enter_context(tc.tile_pool(name="diag", bufs=4))
        pp_prep = ctx.enter_context(tc.tile_pool(name="pprep", bufs=3, space="PSUM"))
        pp_y = ctx.enter_context(tc.tile_pool(name="py", bufs=1, space="PSUM"))

        def psum_prep(name):
            return pp_prep.tile([128, 512], fp32, tag="agg", name=name)

        def ytile(m, wi, name, yc=None):
            # 5 banks: w0/w1 per m; both m's 256-wide w2 halves share the yc
            # bank under a single accumulation group (see zero-mm below)
            if wi == 2:
                return yc[:, m * 256:(m + 1) * 256]
            return pp_y.tile([128, 512], fp32, tag=f"y{m}w{wi}", name=name)

        # ---- persistent small weights
        wpre_d = wpool.tile([CIN, 3, CIN], bf16, tag="wpred")
        for tap in range(3):
            nc.sync.dma_start(out=wpre_d[:, tap, :], in_=wpre_diag_in[tap])
        bpre = wpool.tile([CIN, 1], fp32, tag="bpre")
        nc.sync.dma_start(out=bpre[:], in_=bpre_in[:])
        ident = wpool.tile([128, 128], bf16, tag="ident")
        nc.sync.dma_start(out=ident[:], in_=ident_in[:])
        lam_t = wpool.tile([128, 2, K], fp32, tag="lamt")
        sgn_t = wpool.tile([128, 2, K], fp32, tag="sgnt")
        bsg_t = wpool.tile([128, 2, MH], fp32, tag="bsgt")
        cvs_t = wpool.tile([128, T, K], fp32, tag="cvst")
        nc.sync.dma_start(out=cvs_t[:], in_=cvs_in[:])
        for L in range(2):
            nc.sync.dma_start(out=lam_t[:, L, :], in_=lam_in[L])
            nc.sync.dma_start(out=sgn_t[:, L, :], in_=sgn_in[L])
            nc.sync.dma_start(out=bsg_t[:, L, :], in_=bsg_in[L])
        wmix0 = wpool.tile([128, K2, 2, H], fp8, tag="wmix0")
        wmo = wpool.tile([128, K2, 2, CO], fp8, tag="wmo")
        for k2 in range(K2):
            nc.sync.dma_start(out=wmix0[:, k2, :, :], in_=wmix0_in[k2])
            nc.sync.dma_start(out=wmo[:, k2, :, :], in_=wmo_in[k2])
        wsage = wpool.tile([128, 2, 2, 2, H], bf16, tag="wsage")
        for L in range(2):
            for sn in range(2):
                for cc in range(2):
                    nc.sync.dma_start(out=wsage[:, L, sn, cc, :],
                                      in_=wsage_in[L, sn, cc])
        wres0 = wpool.tile([128, H], bf16, tag="wres0")
        nc.sync.dma_start(out=wres0[:], in_=wres0_in[:])
        wro = wpool.tile([128, 2, CO], bf16, tag="wro")
        for cc in range(2):
            nc.sync.dma_start(out=wro[:, cc, :], in_=wro_in[cc])
        bmr = wpool.tile([128, MH], fp32, tag="bmr")
        nc.sync.dma_start(out=bmr[:], in_=bmr_in[:])
        bout = wpool.tile([64, 1], fp32, tag="bout")
        nc.sync.dma_start(out=bout[:], in_=bout_in[:])
        zot = wpool.tile([128, 512], bf16, tag="zot")
        nc.gpsimd.memset(zot[:], 0)

        # ---- state (bf16); fp32 bitcast doubles as phase-A scratch
        state = spool.tile([128, K * VL], bf16, tag="state")
        st32 = state[:].bitcast(fp32)                      # [128, K*VL//2]
        stv = state[:].rearrange("p (k v) -> p k v", k=K)

        # ---- phase A: token mix (feature-major), stage x0, AllGather per t
        for t in range(T):
            nc.sync.dma_start(out=st32[:, t * VL:(t + 1) * VL], in_=xs_in[t])
        KH = K // 2
        for u in range(T):
            nc.vector.tensor_copy(state[:, (KH + u) * VL:(KH + u + 1) * VL],
                                  st32[:, u * VL:(u + 1) * VL])
        for t in range(T):
            sl = lambda u: state[:, (KH + u) * VL:(KH + u + 1) * VL]
            x0t = mpool.tile([CIN, VL], bf16, tag="x0t")
            yc_a = pp_y.tile([128, 512], fp32, tag="yc", name="yca")
            for wi, (woff, wlen) in enumerate(cfg.VCS):
                tm_ps = ytile(t % 2, wi, "tmps", yc=yc_a)
                taps = [(tap, t + tap - 1) for tap in range(3)
                        if 0 <= t + tap - 1 < T]
                for i, (tap, u) in enumerate(taps):
                    nc.tensor.matmul(
                        out=tm_ps[:, :wlen], lhsT=wpre_d[:, tap, :],
                        rhs=sl(u)[:, woff:woff + wlen],
                        start=(i == 0), stop=(i == len(taps) - 1))
                nc.scalar.activation(x0t[:, woff:woff + wlen],
                                     tm_ps[:, :wlen], AT.Identity,
                                     bias=bpre[:, 0:1], scale=1.0)
            nc.sync.dma_start(out=x0_T[t], in_=x0t[:])
            nm = npool.tile([128, VL // 128, CIN], bf16, tag="nm")
            for bi in range(VL // 128):
                pt = pp_y.tile([128, 512], fp32, tag=f"y{bi % 2}w{(bi // 2) % 2}",
                               name="pt")
                ptb = pt[:].bitcast(bf16)
                nc.tensor.transpose(out=ptb[:, :128],
                                    in_=x0t[:, bi * 128:(bi + 1) * 128],
                                    identity=ident[:])
                nc.vector.tensor_copy(nm[:, bi, :CIN], ptb[:, :CIN])
            nc.sync.dma_start(out=x0_nm[t].rearrange("(b p) c -> p b c", p=128),
                              in_=nm[:])
            if sim1:
                nc.sync.dma_start(out=x0_full[t, :VL, :], in_=x0_nm[t][:])
            else:
                nc.gpsimd.collective_compute(
                    "AllGather", OP.bypass, replica_groups=rg,
                    ins=[x0_nm[t][:]], outs=[x0_full[t][:]])

        # ------------------------------------------------------------------
        # shared per-t one-hot / gather-idx tiles (identical for both layers)
        # ------------------------------------------------------------------
        it_hist = {}

        def load_edges(t):
            if t in it_hist:
                return it_hist[t]
            it = ipool.tile([128, ngrp, GCH * 8], i16, tag="idx", name="it")
            nc.sync.dma_start(
                out=it[:], in_=idx_in[t].rearrange("g p x -> p g x"))
            it_hist[t] = it
            return it

        win_of = {}
        for (woff, wlen) in cfg.VCS:
            for b0 in range(woff // BL, (woff + wlen) // BL):
                win_of[b0] = (woff, wlen)

        # ------------------------------------------------------------------
        # prepare(L, t, xt): gather + segment-sum + SAGE h -> h_sb [128,2,VL]
        # ------------------------------------------------------------------
        def prepare_multi(specs):
            # specs: list of (L, t, xt); chunk loops interleaved so the two
            # gather streams pipeline together
            ctxs = []
            for (L, t, xt) in specs:
                it = load_edges(t)
                mp = m0pool if L == 0 else m1pool
                ctxs.append(dict(
                    L=L, t=t, xt=xt, st=None, it=it,
                    CC=cfg.CC[L], Cin=CIN if L == 0 else H,
                    dt=bf16 if L == 0 else fp8,
                    sdram=s_in if L == 0 else s8_in,
                    xfull=x0_full if L == 0 else x1_full,
                    mean=mp.tile([128, 2, VL], bf16, tag=f"mean{L}",
                                 name="mean_sb"),
                    gt=None, agg=None))
            for ch in range(nchunk):
                g, cg = divmod(ch, GCH)
                b, cb = divmod(ch, cfg.ncb)
                woff, wlen = win_of[b]
                boff = b * BL - woff
                for cx in ctxs:
                    if cg == 0:
                        cx["gt"] = gpool.tile([128, GCH, cx["Cin"]], cx["dt"],
                                              tag=f"g{cx['L']}", name="gt")
                        nc.gpsimd.dma_gather(
                            out_ap=cx["gt"][:], in_ap=cx["xfull"][cx["t"]][:],
                            idxs_ap=cx["it"][:, g, :],
                            num_idxs=GCH * 128, num_idxs_reg=GCH * 128,
                            elem_size=cx["Cin"])
                        cx["st"] = opool.tile([128, GCH, BL], cx["dt"],
                                              tag=f"soh{cx['L']}", name="st")
                        nc.sync.dma_start(
                            out=cx["st"][:],
                            in_=cx["sdram"][cx["t"], :,
                                            g * GCH * BL:(g + 1) * GCH * BL]
                            .rearrange("p (c w) -> p c w", w=BL))
                    if b % (512 // BL) == 0 and cb == 0:
                        cx["agg"] = [psum_prep("agg") for _ in range(cx["CC"])]
                    if cx["L"] == 1 and cfg.ncb % 2 == 0:
                        # fp8 DoubleRow over chunk pairs
                        if cb % 2 == 0:
                            for cc in range(cx["CC"]):
                                nc.tensor.matmul(
                                    out=cx["agg"][cc][:, boff:boff + BL],
                                    lhsT=cx["gt"][:, cg:cg + 2,
                                                  cc * 128:(cc + 1) * 128],
                                    rhs=cx["st"][:, cg:cg + 2, :],
                                    start=(cb == 0),
                                    stop=(cb == cfg.ncb - 2),
                                    perf_mode=PM.DoubleRow)
                    else:
                        for cc in range(cx["CC"]):
                            nc.tensor.matmul(
                                out=cx["agg"][cc][:, boff:boff + BL],
                                lhsT=cx["gt"][:, cg, cc * 128:(cc + 1) * 128],
                                rhs=cx["st"][:, cg, :], start=(cb == 0),
                                stop=(cb == cfg.ncb - 1))
                    if b == (woff + wlen) // BL - 1 and cb == cfg.ncb - 1:
                        for cc in range(cx["CC"]):
                            nc.scalar.activation(
                                cx["mean"][:, cc, woff:woff + wlen],
                                cx["agg"][cc][:, :wlen], AT.Copy)
            outs = []
            for cx in ctxs:
                L, xt, mean_sb = cx["L"], cx["xt"], cx["mean"]
                CC = cx["CC"]
                hp = h0pool if L == 0 else h1pool
                h_sb = hp.tile([128, 2, VL], bf16, tag=f"hsb{L}", name="h_sb")
                for (woff, wlen) in cfg.VCS:
                    for m in range(MH):
                        h_ps = psum_prep("hps")
                        for cc in range(CC):
                            nc.tensor.matmul(
                                out=h_ps[:, :wlen],
                                lhsT=wsage[:, L, 0, cc, m * 128:(m + 1) * 128],
                                rhs=xt[:, cc, woff:woff + wlen],
                                start=(cc == 0), stop=False)
                        for cc in range(CC):
                            nc.tensor.matmul(
                                out=h_ps[:, :wlen],
                                lhsT=wsage[:, L, 1, cc, m * 128:(m + 1) * 128],
                                rhs=mean_sb[:, cc, woff:woff + wlen],
                                start=False, stop=(cc == CC - 1))
                        nc.scalar.activation(
                            h_sb[:, m, woff:woff + wlen], h_ps[:, :wlen],
                            AT.Identity, bias=bsg_t[:, L, m:m + 1], scale=1.0)
                outs.append(h_sb)
            return outs

        def prepare(L, t, xt):
            return prepare_multi([(L, t, xt)])[0]

        def load_x0(t):
            xt = xpool.tile([128, 1, VL], bf16, tag="x0in", name="xt")
            nc.sync.dma_start(out=xt[:, 0, :], in_=x0_T[t, :128, :])
            return xt

        # ------------------------------------------------------------------
        # Layer-0 scan (t-loop) with pipelined layer-1 prepare
        # ------------------------------------------------------------------
        x0t_cur = load_x0(0)
        pre = prepare(0, 0, x0t_cur)
        last_ys = None
        ys_prev = None
        for t in range(T):
            h_sb = pre
            xt = x0t_cur

            # lagged layer-1 prepare for t-1 (x1_full[t-1] ready) and
            # layer-0 prepare for t+1, chunk loops interleaved; both overlap
            # this step's DVE/ACT scan work
            specs = []
            if ys_prev is not None:
                specs.append((1, t - 1, ys_prev))
            if t + 1 < T:
                x0t_cur = load_x0(t + 1)
                specs.append((0, t + 1, x0t_cur))
            if specs:
                outs = prepare_multi(specs)
            if ys_prev is not None:
                h1 = outs[0]
                for m in range(2):
                    nc.sync.dma_start(out=h1_T[t - 1, m], in_=h1[:, m, :])
            if t + 1 < T:
                pre = outs[-1]

            ys = xpool.tile([128, MH, VL], bf16, tag="ys", name="ys")
            yc_t = pp_y.tile([128, 512], fp32, tag="yc", name="yct")
            nc.tensor.matmul(out=yc_t[:], lhsT=ident[:], rhs=zot[:],
                             start=True, stop=False)
            y_ps = {}
            for wi, (woff, wlen) in enumerate(cfg.VCS):
                for m in range(MH):
                    y_ps[(wi, m)] = ytile(m, wi, "yps", yc=yc_t)
                    nc.tensor.matmul(
                        out=y_ps[(wi, m)][:, :wlen],
                        lhsT=wres0[:, m * 128:(m + 1) * 128],
                        rhs=xt[:, 0, woff:woff + wlen],
                        start=(wi != 2), stop=False)
            for k2 in range(K2):
                sv = stv[:, 2 * k2:2 * k2 + 2, :]
                if t == 0:
                    nc.vector.tensor_copy(sv, h_sb[:])
                else:
                    for r in range(2):
                        k = 2 * k2 + r
                        nc.vector.tensor_scalar(
                            sv[:, r, :], sv[:, r, :], lam_t[:, 0, k:k + 1],
                            None, OP.mult)
                    nc.vector.tensor_tensor(out=sv, in0=sv, in1=h_sb[:],
                                            op=OP.add)
                rl = rpool.tile([128, 2, VL], fp8, tag="rl", name="rl")
                for r in range(2):
                    k = 2 * k2 + r
                    if k2 < RSPLIT:
                        nc.scalar.activation(
                            rl[:, r, :], sv[:, r, :], AT.Relu,
                            scale=sgn_t[:, 0, k:k + 1])
                    else:
                        nc.vector.tensor_scalar(
                            rl[:, r, :], sv[:, r, :], sgn_t[:, 0, k:k + 1],
                            0.0, OP.mult, OP.max)
                for wi, (woff, wlen) in enumerate(cfg.VCS):
                    for m in range(MH):
                        nc.tensor.matmul(
                            out=y_ps[(wi, m)][:, :wlen],
                            lhsT=wmix0[:, k2, :, m * 128:(m + 1) * 128],
                            rhs=rl[:, :, woff:woff + wlen],
                            start=False,
                            stop=(k2 == K2 - 1 and wi != 2),
                            perf_mode=PM.DoubleRow)
            nc.tensor.matmul(out=yc_t[:], lhsT=ident[:], rhs=zot[:],
                             start=False, stop=True)
            for wi, (woff, wlen) in enumerate(cfg.VCS):
                for m in range(MH):
                    nc.scalar.activation(
                        ys[:, m, woff:woff + wlen], y_ps[(wi, m)][:, :wlen],
                        AT.Identity, bias=bmr[:, m:m + 1],
                        scale=2.0 ** (-(ER + EW)))
            last_ys = ys

            nm = npool.tile([128, VL // 128, H], fp8, tag="nm8")
            for bi in range(VL // 128):
                pt = pp_y.tile([128, 512], fp32,
                               tag=f"y{bi % 2}w{(bi // 2) % 2}", name="pt")
                ptb = pt[:].bitcast(bf16)
                for m in range(MH):
                    nc.tensor.transpose(
                        out=ptb[:, m * 128:(m + 1) * 128],
                        in_=ys[:, m, bi * 128:(bi + 1) * 128],
                        identity=ident[:])
                nc.vector.tensor_copy(nm[:, bi, :], ptb[:, :H])
            nc.sync.dma_start(
                out=x1_nm[t].rearrange("(b p) c -> p b c", p=128),
                in_=nm[:])
            if sim1:
                nc.sync.dma_start(out=x1_full[t, :VL, :], in_=x1_nm[t][:])
            else:
                nc.gpsimd.collective_compute(
                    "AllGather", OP.bypass, replica_groups=rg,
                    ins=[x1_nm[t][:]], outs=[x1_full[t][:]])

            ys_prev = ys

        # final lagged layer-1 prepare
        h1 = prepare(1, T - 1, ys_prev)
        for m in range(2):
            nc.sync.dma_start(out=h1_T[T - 1, m], in_=h1[:, m, :])

        # ------------------------------------------------------------------
        # Layer-1 tail: s7 = sum_t lam^(7-t) h1_t via PE diag-matmuls,
        # relu -> fp8 -> DoubleRow vs folded w_mix@w_out (+ residual) -> out
        # ------------------------------------------------------------------
        h1a = state[:, :2 * T * VL].rearrange("p (m t v) -> p m t v", m=2, t=T)
        for t in range(T):
            for m in range(2):
                nc.sync.dma_start(out=h1a[:, m, t, :], in_=h1_T[t, m])
        x7 = last_ys                    # layer-0 output at t=7 [128, MH, VL]

        o_tiles = [
            psum_prep("ops0"),
            psum_prep("ops1"),
            pp_y.tile([128, 512], fp32, tag="yc", name="ops2"),
        ]

        def o_slice(wi, wlen):
            return o_tiles[wi][:CO, :wlen]

        for wi, (woff, wlen) in enumerate(cfg.VCS):
            for cc in range(MH):
                nc.tensor.matmul(
                    out=o_slice(wi, wlen), lhsT=wro[:, cc, :],
                    rhs=x7[:, cc, woff:woff + wlen],
                    start=(cc == 0), stop=False)
        NDVE = 7                         # late k2 pairs' s7 on DVE via stt
        for k2 in range(K2):
            on_dve = k2 >= K2 - NDVE // 2 - (NDVE % 2) and False
            rl = rpool.tile([128, 2, VL], fp8, tag="rl", name="rl1")
            if k2 >= K2 - NDVE:
                # s7 accumulated on DVE into the dead upper state region
                for r in range(2):
                    k = 2 * k2 + r
                    m = k % 2
                    t0 = cfg.tail_tmin[k]
                    acc = state[:, (16 + (k2 - (K2 - NDVE)) * 2 + r) * VL:
                                (16 + (k2 - (K2 - NDVE)) * 2 + r + 1) * VL]
                    nc.vector.tensor_scalar(
                        acc, h1a[:, m, t0, :], cvs_t[:, t0, k:k + 1],
                        None, OP.mult)
                    for t in range(t0 + 1, T):
                        nc.vector.scalar_tensor_tensor(
                            out=acc, in0=h1a[:, m, t, :],
                            scalar=cvs_t[:, t, k:k + 1], in1=acc,
                            op0=OP.mult, op1=OP.add)
                    nc.vector.tensor_scalar(
                        rl[:, r, :], acc, sgn_t[:, 1, k:k + 1],
                        0.0, OP.mult, OP.max)
            else:
                diags = {}
                for r in range(2):
                    k = 2 * k2 + r
                    diags[r] = dpool.tile([128, T, 128], bf16, tag="diag",
                                          name="diag")
                    for t in range(cfg.tail_tmin[k], T):
                        nc.vector.tensor_scalar(
                            diags[r][:, t, :], ident[:], cvs_t[:, t, k:k + 1],
                            None, OP.mult)
                for wi, (woff, wlen) in enumerate(cfg.VCS):
                    ps = pp_y.tile([128, 512], fp32,
                                   tag=f"y{k2 % 2}w{wi % 2}", name="s7p")[:, :512]
                    for r in range(2):
                        k = 2 * k2 + r
                        m = k % 2
                        t0 = cfg.tail_tmin[k]
                        for t in range(t0, T):
                            nc.tensor.matmul(
                                out=ps[:, :wlen], lhsT=diags[r][:, t, :],
                                rhs=h1a[:, m, t, woff:woff + wlen],
                                start=(t == t0), stop=(t == T - 1))
                        nc.scalar.activation(
                            rl[:, r, woff:woff + wlen], ps[:, :wlen], AT.Relu,
                            scale=sgn_t[:, 1, k:k + 1])
            for wi, (woff, wlen) in enumerate(cfg.VCS):
                nc.tensor.matmul(
                    out=o_slice(wi, wlen),
                    lhsT=wmo[:, k2, :, :],
                    rhs=rl[:, :, woff:woff + wlen],
                    start=False, stop=(k2 == K2 - 1),
                    perf_mode=PM.DoubleRow)
        for wi, (woff, wlen) in enumerate(cfg.VCS):
            ot = opoolx.tile([CO, 512], fp32, tag="outt")
            nc.scalar.activation(ot[:, :wlen],
                                 o_slice(wi, wlen), AT.Identity,
                                 bias=bout[:, 0:1],
                                 scale=2.0 ** (-(ER + EW)))
            nc.sync.dma_start(out=out_fm[:, woff:woff + wlen], in_=ot[:, :wlen])

    nc.compile()
    return nc


# ----------------------------------------------------------------------------
# host wrapper
# ----------------------------------------------------------------------------

def make_inputs(cfg, inputs):
    T, CIN, H, DS, CO = cfg.T, cfg.CIN, cfg.H, cfg.DS, cfg.CO
    VLOC, VL, K, K2, MH = cfg.VLOC, cfg.VL, cfg.K, cfg.K2, cfg.MH
    xs = np.asarray(inputs["xs"], np.float32)
    src_rows, scol, sval = prep_edges(cfg, inputs["edge_index"])

    w_pre = np.asarray(inputs["w_pre"], np.float32)
    wpre_diag = np.stack([np.diag(w_pre[:, tap]) for tap in range(3)]).astype(BF16)
    bpre = np.asarray(inputs["b_pre"], np.float32).reshape(CIN, 1)
    ident = np.eye(128, dtype=np.float32).astype(BF16)
    nchunk = scol.shape[2]

    lam_a, sgn_a, bsg_a = [], [], []
    wsage_a = []
    wout_f = np.asarray(inputs["w_out"], np.float32)          # [H, CO]
    for L, f in ((0, CIN), (1, H)):
        lam, Bv, wmr = reorder_ssm(inputs[f"a_log{L}"], inputs[f"B{L}"],
                                   inputs[f"w_mix{L}"], H, DS)
        lam_a.append(lam.T)
        # rl = relu(sgn * 2^ER * state); |B| * 2^EW folded into w rows
        sgn_a.append((np.sign(Bv) * 2.0 ** ER).T)
        bsg_a.append(np.asarray(inputs[f"b_sage{L}"], np.float32)
                     .reshape(MH, 128).T)
        wmr = wmr * (np.abs(Bv).reshape(-1)[:, None] * 2.0 ** EW)
        if L == 0:
            wmix0_r = wmr                                     # [K*128, H]
        else:
            wmo_r = wmr @ wout_f                              # [K*128, CO]
            lamp = lam.reshape(-1)
            cvs = np.stack([(lamp ** (T - 1 - t)).reshape(K, 128).T
                            for t in range(T)], 1)            # [128, T, K]
        ws = np.zeros((2, 2, 128, H), np.float32)
        wsf = np.asarray(inputs[f"w_self{L}"], np.float32)
        wnf = np.asarray(inputs[f"w_neigh{L}"], np.float32)
        for cc in range((f + 127) // 128):
            ws[0, cc] = wsf[cc * 128:(cc + 1) * 128]
            ws[1, cc] = wnf[cc * 128:(cc + 1) * 128]
        wsage_a.append(ws.astype(BF16))

    # layer-0 residual: w_res0 prescaled by 2^(ER+EW); ys activation divides
    wres0 = (np.asarray(inputs["w_res0"], np.float32)
             * 2.0 ** (ER + EW)).astype(BF16)
    bmr0 = (np.asarray(inputs["b_res0"], np.float32)
            + np.asarray(inputs["b_mix0"], np.float32)).reshape(MH, 128).T
    # layer-1: residual + biases folded through w_out
    wro = (np.asarray(inputs["w_res1"], np.float32) @ wout_f
           * 2.0 ** (ER + EW)).reshape(MH, 128, CO)
    bo = ((np.asarray(inputs["b_res1"], np.float32)
           + np.asarray(inputs["b_mix1"], np.float32)) @ wout_f
          + np.asarray(inputs["b_out"], np.float32))          # natural units

    for k in range(K):
        keep = np.max(np.abs(cvs[:, :, k]), axis=0) >= 3e-3   # [T]
        t0 = 0
        while t0 < T - 1 and not keep[t0]:
            t0 += 1
        cfg.tail_tmin[k] = t0

    wmix0_q = np.zeros((K2, 128, 2, H), FP8)
    wmo_q = np.zeros((K2, 128, 2, CO), FP8)
    for k2 in range(K2):
        for r in range(2):
            k = 2 * k2 + r
            wmix0_q[k2, :, r, :] = wmix0_r[k * 128:(k + 1) * 128].astype(FP8)
            wmo_q[k2, :, r, :] = wmo_r[k * 128:(k + 1) * 128].astype(FP8)

    wcol = np.arange(BL, dtype=np.float32)

    in_maps = []
    for c in range(NCORES):
        oh = (scol[c][..., None] == wcol) * sval[c][..., None]  # [T, nchunk, 128, BL]
        s_tiles = np.ascontiguousarray(
            oh.transpose(0, 2, 1, 3).reshape(T, 128, nchunk * BL)).astype(BF16)
        s8_tiles = s_tiles.astype(FP8)
        sh = xs[:, c * VLOC:(c + 1) * VLOC, :]
        xs_sh = np.zeros((T, CIN, VL), np.float32)
        xs_sh[:, :, :VLOC] = np.transpose(sh, (0, 2, 1))
        in_maps.append({
            "xs_in": xs_sh,
            "idx_in": pack_gather_idx(cfg, src_rows[c]),
            "s_in": s_tiles,
            "s8_in": s8_tiles,
            "wpre_diag_in": wpre_diag,
            "bpre_in": bpre,
            "ident_in": ident,
            "lam_in": np.stack(lam_a).astype(np.float32),
            "sgn_in": np.stack(sgn_a).astype(np.float32),
            "bsg_in": np.stack(bsg_a).astype(np.float32),
            "cvs_in": cvs.astype(np.float32),
            "wmix0_in": wmix0_q,
            "wmo_in": wmo_q,
            "wsage_in": np.stack(wsage_a),
            "wres0_in": wres0,
            "wro_in": wro.astype(BF16),
            "bmr_in": bmr0.astype(np.float32),
            "bout_in": bo.reshape(CO, 1).astype(np.float32),
        })
    return in_maps


_CACHED = {}


def kernel(**inputs):
    cfg = Cfg()
    in_maps = make_inputs(cfg, inputs)
    key = ("full", cfg.ncb)
    if key not in _CACHED:
        _CACHED[key] = build_program(cfg)
    nc = _CACHED[key]
    res = run_bass_kernel_spmd(nc, in_maps, list(range(NCORES)))
    out = np.zeros((cfg.V, cfg.CO), np.float32)
    for c in range(NCORES):
        out[c * cfg.VLOC:(c + 1) * cfg.VLOC] = \
            res.results[c]["out_fm"][:, :cfg.VLOC].T
    return out
